# revision 21
# baseline (speedup 1.0000x reference)
"""Trainium2 Bass kernel for protein-feature GNN message passing.

Sharding: data-parallel over 16 chains -> 8 cores x 2 chains.
Per chain (L=2048): KNN top-9 via fp32r matmul (2a.b - |b|^2, monotone in -d^2
per row) -> per-quarter top-8 candidates (DVE max8/max_index) -> exact fp32
re-rank of 32 candidates -> node/edge features on ACT/GPSIMD -> DMA out.
"""
import sys
sys.path.insert(0, '/opt/trn_rl_repo')
import numpy as np

import inspect as _inspect
import concourse.bass as bass
import concourse.bacc as bacc
import concourse.mybir as mybir
from concourse import tile
from concourse.bass import AP
from concourse.bass_utils import run_bass_kernel_spmd

# dma_gather's 256B elem assert is a transpose-mode restriction; small payloads
# with 256B row STRIDE work (HW-verified). Patch the assert.
_gsrc = _inspect.getsource(bass.BassGpSimd.dma_gather)
_gsrc = _gsrc.replace("elem_size_bytes > 0 and elem_size_bytes % 256 == 0",
                      "elem_size_bytes > 0")
_gns = dict(bass.__dict__)
exec("def _patched" + _gsrc[len("    def dma_gather"):].replace("\n    ", "\n"), _gns)
bass.BassGpSimd.dma_gather = _gns["_patched"]

dt = mybir.dt
AF = mybir.ActivationFunctionType
ALU = mybir.AluOpType
AX = mybir.AxisListType

L = 2048
NBLK = 16
CH = 2
NCORE = 8
KNN = 9
NCAND = 32
E = 16 * KNN              # 144 edge slots per partition
EPS = 1e-6
SIGMA = 20.0 / 16.0
TWO_PI = float(2.0 * np.pi)
HALF_PI = float(np.pi / 2.0)
PI = float(np.pi)
MUS = np.linspace(0.0, 20.0, 16).astype(np.float32)
POS_FREQ = np.exp(np.arange(0, 16, 2, dtype=np.float32) *
                  (-np.log(10000.0) / 16)).astype(np.float32)

A_N, A_CA, A_C, A_O, A_SEG = 0, 3, 6, 9, 12


def V(t, off, dims):
    """Free-dim view on an SBUF tile (keeps partition dim)."""
    return AP(t.tensor, t.offset + off, [list(t.ap[0])] + [list(d) for d in dims])


def PV(t, poff, dims):
    """View with partition offset (for the 16-wide wrap tiles)."""
    ap0 = list(t.ap[0])
    return AP(t.tensor, t.offset + poff, [ap0] + [list(d) for d in dims])


def DV(d, off, dims):
    return AP(d.tensor, d.offset + off, [list(x) for x in dims])


class _Chain:
    def __init__(self, nc, tc, pools, io, c, gsem, gcnt):
        self.nc = nc
        self.tc = tc
        self.gsem = gsem
        self.gcnt = gcnt
        self.sb = pools['sb']
        self.big = pools['big']
        self.huge = pools['huge']
        self.ps = pools['ps']
        self.io = io
        self.c = c

    def t(self, shape, dtype, tag, bufs=None):
        return self.sb.tile(shape, dtype, tag=tag, name=tag or "tmp", bufs=bufs)

    def bt(self, shape, dtype, tag, bufs=None):
        return self.big.tile(shape, dtype, tag=tag, name=tag or "tmp", bufs=bufs)

    def ht(self, shape, dtype, tag):
        return self.huge.tile(shape, dtype, tag=tag, name=tag or "tmp")

    # ---------- helpers ----------
    def norm3(self, u, tag, out_ap=None):
        """normalize [128,16,3]: u * rsqrt(max(sumsq, 1e-24))."""
        nc = self.nc
        squ = self.t([128, 16, 3], dt.float32, "h_sq")
        nc.scalar.activation(squ[:], u[:], AF.Square)
        ss = self.t([128, 16], dt.float32, "h_ss")
        nc.vector.tensor_reduce(ss[:], squ[:], AX.X, ALU.add)
        nr = self.t([128, 16], dt.float32, "h_nr")
        nc.scalar.activation(nr[:], ss[:], AF.Sqrt)
        nc.gpsimd.tensor_scalar(nr[:], nr[:], 1e-12, None, ALU.max)
        rs = self.t([128, 16], dt.float32, "h_rs")
        nc.vector.reciprocal(rs[:], nr[:])
        if out_ap is None:
            un = self.t([128, 16, 3], dt.float32, tag)
            out_ap = un[:]
        else:
            un = None
        nc.gpsimd.tensor_tensor(out_ap, u[:], V(rs, 0, [[1, 16], [0, 3]]), ALU.mult)
        return un

    def rot(self, u, which, tag):
        nc = self.nc
        r = self.t([128, 16, 3], dt.float32, tag)
        if which == 1:      # (y, z, x)
            nc.scalar.copy(V(r, 0, [[3, 16], [1, 2]]), V(u, 1, [[3, 16], [1, 2]]))
            nc.scalar.copy(V(r, 2, [[3, 16], [1, 1]]), V(u, 0, [[3, 16], [1, 1]]))
        else:               # (z, x, y)
            nc.scalar.copy(V(r, 0, [[3, 16], [1, 1]]), V(u, 2, [[3, 16], [1, 1]]))
            nc.scalar.copy(V(r, 1, [[3, 16], [1, 2]]), V(u, 0, [[3, 16], [1, 2]]))
        return r

    def cross(self, tag, a1, a2, b1, b2):
        nc = self.nc
        m1 = self.t([128, 16, 3], dt.float32, "h_m1")
        nc.gpsimd.tensor_tensor(m1[:], a1[:], b2[:], ALU.mult)
        m2 = self.t([128, 16, 3], dt.float32, "h_m2")
        nc.gpsimd.tensor_tensor(m2[:], a2[:], b1[:], ALU.mult)
        w = self.t([128, 16, 3], dt.float32, tag)
        nc.gpsimd.tensor_tensor(w[:], m1[:], m2[:], ALU.subtract)
        return w

    def dot16(self, a_ap, b_ap, out_ap):
        nc = self.nc
        m = self.t([128, 16, 3], dt.float32, "h_dm")
        nc.gpsimd.tensor_tensor(m[:], a_ap, b_ap, ALU.mult)
        nc.vector.tensor_reduce(out_ap, m[:], AX.X, ALU.add)

    def wrap_idx(self, src_u32_ap, width, tag):
        """[128, width] u32 AP -> replicated wrapped int16 idx tile [128, 8*width]."""
        nc = self.nc
        wr32 = self.t([16, 8 * 8 * NCAND], dt.uint32, "wrap32", bufs=1)
        for ph in range(8):
            nc.sync.dma_start(PV(wr32, ph, [[8, width]]),
                              AP(src_u32_ap.tensor,
                                 src_u32_ap.offset + ph * 16 * src_u32_ap.ap[0][0],
                                 [[src_u32_ap.ap[0][0], 16]] + src_u32_ap.ap[1:]))
        wr = self.t([16, 8 * 8 * NCAND], dt.int16, "wrap16", bufs=1)
        nc.gpsimd.tensor_copy(wr[:, :8 * width], wr32[:, :8 * width])
        wrf = self.t([128, 8 * 8 * NCAND], dt.int16, "wrapf", bufs=2)
        for g in range(8):
            nc.sync.dma_start(wrf[g * 16:(g + 1) * 16, :8 * width],
                              wr[:, :8 * width])
        return wrf

    def gather(self, out_ap, in_ap, wrf, nidx, elem_size, elem_step):
        nc = self.nc
        with self.tc.tile_critical():
            nc.gpsimd.dma_gather(out_ap, in_ap, wrf[:], nidx, nidx, elem_size,
                                 elem_step=elem_step,
                                 single_packet=False).then_inc(self.gsem, 16)
            self.gcnt[0] += 16
            nc.gpsimd.wait_ge(self.gsem, self.gcnt[0])

    def gather_by(self, src_tile_ap_fn, out_fn, in_ap, total_w, elem_size,
                  elem_step, tag, chunk=64):
        """Chunked gather: per chunk of width<=64 idx columns, wrap + gather."""
        off = 0
        while off < total_w:
            w = min(chunk, total_w - off)
            wrf = self.wrap_idx(src_tile_ap_fn(off, w), w, tag)
            self.gather(out_fn(off, w), in_ap, wrf, w * 128, elem_size, elem_step)
            off += w

    # ---------- phases ----------
    def setup(self):
        nc, io, c = self.nc, self.io, self.c
        self.xp = self.t([4, L], dt.float32, "ptmp", bufs=1)
        nc.sync.dma_start(self.xp[:], DV(io['xplanes'], c * 4 * L, [[L, 4], [1, L]]))
        self.xpr = self.t([4, L], dt.float32r, "xpr", bufs=1)
        nc.scalar.activation(self.xpr[:], self.xp[:], AF.Copy)
        sq = self.t([3, L], dt.float32r, "ptmp", bufs=1)
        nc.scalar.activation(sq[:], self.xpr[0:3, :].bitcast(dt.float32), AF.Square)
        ones31 = self.t([3, 1], dt.float32r, "ones31")
        nc.gpsimd.memset(ones31[:].bitcast(dt.float32), -1.0)
        rr = self.ps.tile([128, L], dt.float32, tag="dmat", name="rr")
        for q in range(4):
            nc.tensor.matmul(rr[0:1, q * 512:(q + 1) * 512], ones31[:],
                             sq[:, q * 512:(q + 1) * 512])
        self.v4 = self.t([4, L], dt.float32r, "v4", bufs=1)
        nc.scalar.activation(self.v4[0:3, :], self.xpr[0:3, :].bitcast(dt.float32),
                             AF.Copy, scale=2.0)
        rtmp = self.t([1, L], dt.float32, "rtmp", bufs=1)
        nc.scalar.activation(rtmp[:], rr[0:1, :], AF.Copy)
        nc.sync.dma_start(self.v4[3:4, :].bitcast(dt.float32), rtmp[:])

        base = c * L * 16
        self.xr0 = self.t([128, 16, 16], dt.float32, "xr0")
        nc.sync.dma_start(self.xr0[:],
                          DV(io['xrows'], base, [[16, 128], [128 * 16, 16], [1, 16]]))
        self.xrm = self.t([128, 16, 16], dt.float32, "xrm")
        nc.gpsimd.memset(self.xrm[:], 0.0)
        nc.sync.dma_start(self.xrm[1:128, 0:1, :],
                          DV(io['xrows'], base, [[16, 127], [1, 16]]))
        nc.sync.dma_start(self.xrm[:, 1:16, :],
                          DV(io['xrows'], base + 127 * 16,
                             [[16, 128], [128 * 16, 15], [1, 16]]))
        self.xrp = self.t([128, 16, 16], dt.float32, "xrp")
        nc.gpsimd.memset(self.xrp[:], 0.0)
        nc.sync.dma_start(self.xrp[:, 0:15, :],
                          DV(io['xrows'], base + 16,
                             [[16, 128], [128 * 16, 15], [1, 16]]))
        nc.sync.dma_start(self.xrp[0:127, 15:16, :],
                          DV(io['xrows'], base + 1921 * 16, [[16, 127], [1, 16]]))
        self.mu = self.t([128, 16], dt.float32, "mu")
        nc.sync.dma_start(self.mu[:], io['murow'][:])

    def topk(self):
        nc = self.nc
        self.cidx = self.t([128, 16, NCAND], dt.uint32, "cidx")
        for b in range(NBLK):
            dmat = self.ps.tile([128, L], dt.float32, tag="dmat", name="dmat")
            for q in range(4):
                nc.tensor.matmul(dmat[:, q * 512:(q + 1) * 512],
                                 self.xpr[:, b * 128:(b + 1) * 128],
                                 self.v4[:, q * 512:(q + 1) * 512])
            for q in range(4):
                vq = self.t([128, 8], dt.float32, "vq")
                win = dmat[:, q * 512:(q + 1) * 512]
                nc.vector.max(vq[:], win)
                nc.vector.max_index(V(self.cidx, b * NCAND + q * 8, [[1, 8]]),
                                    vq[:], win)

    def refine(self):
        import os as _os
        refk = int(_os.environ.get("REFK", "9"))
        nc, io, c = self.nc, self.io, self.c
        qoff = self.t([128, 16, NCAND], dt.uint32, "qoff", bufs=1)
        nc.gpsimd.iota(V(qoff, 0, [[NCAND, 16], [8, 4], [1, 8]]),
                       pattern=[[0, 16], [512, 4], [0, 8]], base=0,
                       channel_multiplier=0)
        nc.gpsimd.tensor_tensor(self.cidx[:], self.cidx[:], qoff[:], ALU.add)
        self.cg = self.cidx
        ccand = self.bt([128, 16 * NCAND, 3], dt.float32, "ccand", bufs=1)
        if refk < 2:
            nc.gpsimd.memset(ccand[:], 1.0)
        else:
            self.gather_by(
                lambda o, w: V(self.cg, o, [[1, w]]),
                lambda o, w: V(ccand, o * 3, [[3, w], [1, 3]]),
                DV(io['catab'], c * L * 64, [[64, L], [1, 3]]),
                16 * NCAND, 3, 64, "wrc")
        if refk < 3:
            nc.gpsimd.memset(V(ccand, 0, [[1, 100]]), 0.5)
        import os as _os3
        if _os3.environ.get("DBGCG", "0") == "1":
            nc.sync.dma_start(DV(io['col9'], c * L * KNN,
                                 [[KNN, 128], [128 * KNN, 16], [1, KNN]]),
                              V(self.cg, 0, [[NCAND, 16], [1, KNN]]))
        dview = V(ccand, 0, [[NCAND * 3, 16], [3, NCAND], [1, 3]])
        nc.gpsimd.tensor_tensor(dview, dview,
                                V(self.xr0, A_CA, [[16, 16], [0, NCAND], [1, 3]]),
                                ALU.subtract)
        nc.scalar.activation(dview, dview, AF.Square)
        self.negd2 = self.t([128, 16, NCAND], dt.float32, "negd2")
        nc.vector.tensor_reduce(self.negd2[:], dview, AX.X, ALU.add)
        nc.gpsimd.tensor_scalar(self.negd2[:], self.negd2[:], -1.0, None, ALU.mult)
        self.pos = self.t([128, 16, 16], dt.uint32, "pos")
        for b in range(NBLK):
            nv = V(self.negd2, b * NCAND, [[1, NCAND]])
            v8 = self.t([128, 8], dt.float32, "rv8")
            nc.vector.max(v8[:], nv)
            mr = self.t([128, NCAND], dt.float32, "rmr")
            nc.vector.match_replace(mr[:], v8[:], nv, -1e30)
            v9 = self.t([128, 8], dt.float32, "rv9")
            nc.vector.max(v9[:], mr[:])
            nc.vector.max_index(V(self.pos, b * 16, [[1, 8]]), v8[:], nv)
            nc.vector.max_index(V(self.pos, b * 16 + 8, [[1, 8]]), v9[:], mr[:])

    def col_extract(self):
        """col[p,b,k] = cg[p,b,pos[p,b,k]] via one-hot eq-select."""
        nc, io, c = self.nc, self.io, self.c
        iotac = self.t([128, NCAND], dt.uint32, "iotac")
        nc.gpsimd.iota(iotac[:], pattern=[[1, NCAND]], base=0, channel_multiplier=0)
        cgf = self.t([128, 16, NCAND], dt.float32, "cgf")
        nc.vector.tensor_copy(cgf[:], self.cg[:])
        colf = self.t([128, 16, KNN], dt.float32, "colf")
        HC = NCAND // 2
        for hf in range(2):
            mask = self.bt([128, 16, KNN, HC], dt.float32, "emask", bufs=1)
            nc.vector.tensor_tensor(mask[:],
                                    V(iotac, hf * HC, [[0, 16], [0, KNN], [1, HC]]),
                                    V(self.pos, 0, [[16, 16], [1, KNN], [0, HC]]),
                                    ALU.is_equal)
            nc.gpsimd.tensor_tensor(mask[:], mask[:],
                                    V(cgf, hf * HC, [[NCAND, 16], [0, KNN], [1, HC]]),
                                    ALU.mult)
            if hf == 0:
                nc.vector.tensor_reduce(colf[:], mask[:], AX.X, ALU.add)
            else:
                half2 = self.t([128, 16, KNN], dt.float32, "colf2")
                nc.vector.tensor_reduce(half2[:], mask[:], AX.X, ALU.add)
                nc.gpsimd.tensor_tensor(colf[:], colf[:], half2[:], ALU.add)
        self.col = self.t([128, E], dt.uint32, "col")
        nc.gpsimd.tensor_copy(V(self.col, 0, [[KNN, 16], [1, KNN]]), colf[:])
        nc.sync.dma_start(DV(io['col9'], c * L * KNN,
                             [[KNN, 128], [128 * KNN, 16], [1, KNN]]),
                          V(self.col, 0, [[KNN, 16], [1, KNN]]))

    def node_feats(self):
        nc, io, c = self.nc, self.io, self.c
        x0, xm, xp_ = self.xr0, self.xrm, self.xrp

        def sl(t, off):
            return V(t, off, [[16, 16], [1, 3]])

        # ---- dis_node ----
        disn = self.bt([128, 16, 48], dt.float32, "disn", bufs=1)
        self.ndiff = {}
        for pi, off in enumerate((A_N, A_C, A_O)):
            d3 = self.t([128, 16, 3], dt.float32, f"nd{pi}")
            nc.gpsimd.tensor_tensor(d3[:], sl(x0, off), sl(x0, A_CA), ALU.subtract)
            self.ndiff[off] = d3
            s3 = self.t([128, 16, 3], dt.float32, "h_sq")
            nc.scalar.activation(s3[:], d3[:], AF.Square)
            d2 = self.t([128, 16], dt.float32, "h_ss")
            nc.vector.tensor_reduce(d2[:], s3[:], AX.X, ALU.add)
            dd = self.t([128, 16], dt.float32, "ndd")
            nc.scalar.activation(dd[:], d2[:], AF.Sqrt, bias=EPS)
            um = self.t([128, 16, 16], dt.float32, "num")
            nc.gpsimd.tensor_tensor(um[:], V(dd, 0, [[1, 16], [0, 16]]),
                                    V(self.mu, 0, [[0, 16], [1, 16]]), ALU.subtract)
            nc.scalar.activation(um[:], um[:], AF.Square, scale=1.0 / SIGMA)
            nc.scalar.activation(V(disn, pi * 16, [[48, 16], [1, 16]]), um[:],
                                 AF.Exp, scale=-1.0)
        nc.sync.dma_start(DV(io['disn'], c * L * 48,
                             [[48, 128], [128 * 48, 16], [1, 48]]),
                          V(disn, 0, [[48, 16], [1, 48]]))

        # ---- angles ----
        ua = self.t([128, 16, 3], dt.float32, "ua")
        nc.gpsimd.tensor_tensor(ua[:], sl(x0, A_N), sl(xm, A_C), ALU.subtract)
        ub = self.t([128, 16, 3], dt.float32, "ub")
        nc.gpsimd.tensor_tensor(ub[:], sl(x0, A_CA), sl(x0, A_N), ALU.subtract)
        uc = self.t([128, 16, 3], dt.float32, "uc")
        nc.gpsimd.tensor_tensor(uc[:], sl(x0, A_C), sl(x0, A_CA), ALU.subtract)
        ud = self.t([128, 16, 3], dt.float32, "ud")
        nc.gpsimd.tensor_tensor(ud[:], sl(xp_, A_N), sl(x0, A_C), ALU.subtract)
        ue = self.t([128, 16, 3], dt.float32, "ue")
        nc.gpsimd.tensor_tensor(ue[:], sl(xp_, A_CA), sl(xp_, A_N), ALU.subtract)
        us = [self.norm3(u, f"un{i}") for i, u in enumerate((ua, ub, uc, ud, ue))]
        rots = {}
        for i, u in enumerate(us):
            rots[(i, 1)] = self.rot(u, 1, f"ur{i}1")
            rots[(i, 2)] = self.rot(u, 2, f"ur{i}2")
        ws = []
        for i in range(4):
            w = self.cross(f"wx{i}", rots[(i, 1)], rots[(i, 2)],
                           rots[(i + 1, 1)], rots[(i + 1, 2)])
            ws.append(self.norm3(w, f"wn{i}"))
        ang = self.bt([128, 16, 12], dt.float32, "angn")
        cosd = self.t([128, 16, 3], dt.float32, "cosd")
        sgn = self.t([128, 16, 3], dt.float32, "sgn")
        cosa = self.t([128, 16, 3], dt.float32, "cosa")
        for ti in range(3):
            self.dot16(ws[ti][:], ws[ti + 1][:], V(cosd, ti, [[3, 16]]))
            self.dot16(us[ti][:], ws[ti + 1][:], V(sgn, ti, [[3, 16]]))
            self.dot16(us[ti][:], us[ti + 1][:], V(cosa, ti, [[3, 16]]))
        nc.scalar.activation(sgn[:], sgn[:], AF.Sign)
        CL = 1.0 - EPS
        nc.gpsimd.tensor_scalar(V(ang, 0, [[12, 16], [1, 3]]), cosd[:], CL, -CL,
                                ALU.min, ALU.max)
        nc.gpsimd.tensor_scalar(V(ang, 6, [[12, 16], [1, 3]]), cosa[:], CL, -CL,
                                ALU.min, ALU.max)
        for bse, sg in ((0, sgn), (6, None)):
            s2 = self.t([128, 16, 3], dt.float32, "s2")
            nc.scalar.activation(s2[:], V(ang, bse, [[12, 16], [1, 3]]), AF.Square)
            nc.gpsimd.tensor_scalar(s2[:], s2[:], -1.0, 1.0, ALU.mult, ALU.add)
            sr = self.t([128, 16, 3], dt.float32, "sr")
            nc.scalar.activation(sr[:], s2[:], AF.Sqrt)
            if sg is not None:
                nc.gpsimd.tensor_tensor(V(ang, bse + 3, [[12, 16], [1, 3]]), sr[:],
                                        sg[:], ALU.mult)
            else:
                nc.scalar.copy(V(ang, bse + 3, [[12, 16], [1, 3]]), sr[:])
        angm = self.t([128, 16, 12], dt.float32, "angm")
        nc.sync.dma_start(angm[:], DV(io['angmul'], c * 128 * 192,
                                      [[192, 128], [12, 16], [1, 12]]))
        anga = self.t([128, 16, 12], dt.float32, "anga")
        nc.sync.dma_start(anga[:], DV(io['angadd'], c * 128 * 192,
                                      [[192, 128], [12, 16], [1, 12]]))
        nc.gpsimd.tensor_tensor(ang[:], ang[:], angm[:], ALU.mult)
        nc.gpsimd.tensor_tensor(ang[:], ang[:], anga[:], ALU.add)
        nc.sync.dma_start(DV(io['angn'], c * L * 12,
                             [[12, 128], [128 * 12, 16], [1, 12]]),
                          V(ang, 0, [[12, 16], [1, 12]]))

        # ---- frames (into one oall [128,16,9] tile: o1,nf,o2) ----
        dA = self.t([128, 16, 3], dt.float32, "dA")
        nc.gpsimd.tensor_tensor(dA[:], sl(x0, A_CA), sl(xm, A_CA), ALU.subtract)
        ucA = self.norm3(dA, "ucA")
        dB = self.t([128, 16, 3], dt.float32, "dB")
        nc.gpsimd.tensor_tensor(dB[:], sl(xp_, A_CA), sl(x0, A_CA), ALU.subtract)
        ucB = self.norm3(dB, "ucB")
        nfr = self.cross("nfx", self.rot(ucA, 1, "ra1"), self.rot(ucA, 2, "ra2"),
                         self.rot(ucB, 1, "rb1"), self.rot(ucB, 2, "rb2"))
        nf = self.norm3(nfr, "nf")
        do1 = self.t([128, 16, 3], dt.float32, "do1")
        nc.gpsimd.tensor_tensor(do1[:], ucA[:], ucB[:], ALU.subtract)
        o1 = self.norm3(do1, "o1")
        o2 = self.cross("o2x", self.rot(o1, 1, "ro1"), self.rot(o1, 2, "ro2"),
                        self.rot(nf, 1, "rn1"), self.rot(nf, 2, "rn2"))
        fm = self.t([128, 16], dt.float32, "fm")
        nc.sync.dma_start(fm[:], io['framemask'][:])
        fmb = V(fm, 0, [[1, 16], [0, 3]])
        self.oall = self.bt([128, 16, 9], dt.float32, "oall")
        nc.gpsimd.tensor_tensor(V(self.oall, 0, [[9, 16], [1, 3]]), o1[:], fmb,
                                ALU.mult)
        nc.gpsimd.tensor_tensor(V(self.oall, 3, [[9, 16], [1, 3]]), nf[:], fmb,
                                ALU.mult)
        nc.gpsimd.tensor_tensor(V(self.oall, 6, [[9, 16], [1, 3]]), o2[:], fmb,
                                ALU.mult)
        rec = self.bt([128, 16, 16], dt.float32, "rec")
        nc.scalar.copy(V(rec, 0, [[16, 16], [1, 3]]), sl(x0, A_CA))
        nc.scalar.copy(V(rec, 3, [[16, 16], [1, 9]]), V(self.oall, 0, [[9, 16], [1, 9]]))
        nc.scalar.copy(V(rec, 12, [[16, 16], [1, 1]]), V(x0, A_SEG, [[16, 16], [1, 1]]))
        nc.gpsimd.memset(V(rec, 13, [[16, 16], [1, 3]]), 0.0)
        nc.sync.dma_start(DV(io['rectab'], c * L * 64,
                             [[64, 128], [128 * 64, 16], [1, 16]]),
                          V(rec, 0, [[16, 16], [1, 16]]))

        # ---- direct_node ----
        dirn = self.bt([128, 16, 9], dt.float32, "dirn")
        for ai, off in enumerate((A_N, A_C, A_O)):
            dv = self.t([128, 16, 3], dt.float32, "dv")
            for ri in range(3):
                self.dot16(V(self.oall, 3 * ri, [[9, 16], [1, 3]]),
                           self.ndiff[off][:], V(dv, ri, [[3, 16]]))
            self.norm3(dv, "", out_ap=V(dirn, ai * 3, [[9, 16], [1, 3]]))
        nc.sync.dma_start(DV(io['dirn'], c * L * 9,
                             [[9, 128], [128 * 9, 16], [1, 9]]),
                          V(dirn, 0, [[9, 16], [1, 9]]))

    def edge_feats(self):
        nc, io, c = self.nc, self.io, self.c
        recq = self.ht([128, E, 16], dt.float32, "recq")
        self.gather_by(
            lambda o, w: V(self.col, o, [[1, w]]),
            lambda o, w: V(recq, o * 16, [[16, w], [1, 16]]),
            DV(io['rectab'], c * L * 64, [[64, L], [1, 16]]),
            E, 16, 64, "wrr")

        def rq(off, n=3):
            return V(recq, off, [[16 * KNN, 16], [16, KNN], [1, n]])

        def x0b(off, n=3):
            return V(self.xr0, off, [[16, 16], [0, KNN], [1, n]])

        # ---- dis_edge (atom-major DRAM; host reorders columns) ----
        self.ediffs = {}
        for ai, off in enumerate((A_N, A_CA, A_C, A_O)):
            de3 = self.bt([128, 16, KNN, 3], dt.float32, f"ed{ai}", bufs=1)
            nc.gpsimd.tensor_tensor(de3[:], x0b(off), rq(0), ALU.subtract)
            self.ediffs[off] = de3
            se3 = self.bt([128, 16, KNN, 3], dt.float32, "eds", bufs=1)
            nc.scalar.activation(se3[:], de3[:], AF.Square)
            e2 = self.t([128, 16, KNN], dt.float32, "ed2")
            nc.vector.tensor_reduce(e2[:], se3[:], AX.X, ALU.add)
            ed = self.t([128, 16, KNN], dt.float32, "edd")
            nc.scalar.activation(ed[:], e2[:], AF.Sqrt, bias=EPS)
            ue = self.ht([128, 16, KNN, 16], dt.float32, "eum")
            nc.gpsimd.tensor_tensor(ue[:], V(ed, 0, [[KNN, 16], [1, KNN], [0, 16]]),
                                    V(self.mu, 0, [[0, 16], [0, KNN], [1, 16]]),
                                    ALU.subtract)
            nc.scalar.activation(ue[:], ue[:], AF.Square, scale=1.0 / SIGMA)
            nc.scalar.activation(ue[:], ue[:], AF.Exp, scale=-1.0)
            nc.sync.dma_start(
                DV(io['dise'], (c * 4 + ai) * L * KNN * 16,
                   [[KNN * 16, 128], [128 * KNN * 16, 16], [1, KNN * 16]]),
                V(ue, 0, [[KNN * 16, 16], [1, KNN * 16]]))

        # ---- angle_edge: R_ij = sum_k Or[k,i] * Oc[k,j] ----
        R = [[None] * 3 for _ in range(3)]
        for i in range(3):
            ob = V(self.oall, i, [[9, 16], [0, KNN], [3, 3]])
            for j in range(3):
                m = self.bt([128, 16, KNN, 3], dt.float32, "rm", bufs=1)
                nc.gpsimd.tensor_tensor(
                    m[:], ob, V(recq, 3 + j, [[16 * KNN, 16], [16, KNN], [3, 3]]),
                    ALU.mult)
                rij = self.t([128, 16, KNN], dt.float32, f"rr{i}{j}")
                nc.vector.tensor_reduce(rij[:], m[:], AX.X, ALU.add)
                R[i][j] = rij
        quat = self.bt([128, 16, KNN, 4], dt.float32, "quat", bufs=1)
        mags = self.bt([128, 16, KNN, 3], dt.float32, "qmags", bufs=1)
        for k in range(3):
            t2 = self.t([128, 16, KNN], dt.float32, "qt")
            if k == 0:
                nc.gpsimd.tensor_tensor(t2[:], R[0][0][:], R[1][1][:], ALU.subtract)
                nc.gpsimd.tensor_tensor(t2[:], t2[:], R[2][2][:], ALU.subtract)
            elif k == 1:
                nc.gpsimd.tensor_tensor(t2[:], R[1][1][:], R[0][0][:], ALU.subtract)
                nc.gpsimd.tensor_tensor(t2[:], t2[:], R[2][2][:], ALU.subtract)
            else:
                nc.gpsimd.tensor_tensor(t2[:], R[2][2][:], R[0][0][:], ALU.subtract)
                nc.gpsimd.tensor_tensor(t2[:], t2[:], R[1][1][:], ALU.subtract)
            nc.scalar.activation(t2[:], t2[:], AF.Abs, bias=1.0)
            nc.scalar.activation(V(mags, k, [[KNN * 3, 16], [3, KNN], [1, 1]]),
                                 t2[:], AF.Sqrt, scale=0.25, bias=0.25e-12)
        sgns = self.bt([128, 16, KNN, 3], dt.float32, "qsgn", bufs=1)
        for k, (i, j) in enumerate(((2, 1), (0, 2), (1, 0))):
            nc.gpsimd.tensor_tensor(V(sgns, k, [[KNN * 3, 16], [3, KNN], [1, 1]]),
                                    R[i][j][:], R[j][i][:], ALU.subtract)
        nc.scalar.activation(sgns[:], sgns[:], AF.Sign)
        nc.gpsimd.tensor_tensor(V(quat, 0, [[KNN * 4, 16], [4, KNN], [1, 3]]),
                                sgns[:], mags[:], ALU.mult)
        tr = self.t([128, 16, KNN], dt.float32, "qtr")
        nc.gpsimd.tensor_tensor(tr[:], R[0][0][:], R[1][1][:], ALU.add)
        nc.gpsimd.tensor_tensor(tr[:], tr[:], R[2][2][:], ALU.add)
        nc.scalar.activation(tr[:], tr[:], AF.Relu, bias=1.0)
        nc.scalar.activation(V(quat, 3, [[KNN * 4, 16], [4, KNN], [1, 1]]), tr[:],
                             AF.Sqrt, scale=0.25, bias=0.25e-12)
        qs = self.bt([128, 16, KNN, 4], dt.float32, "qsq", bufs=1)
        nc.scalar.activation(qs[:], quat[:], AF.Square)
        qss = self.t([128, 16, KNN], dt.float32, "qss")
        nc.vector.tensor_reduce(qss[:], qs[:], AX.X, ALU.add)
        nc.scalar.activation(qss[:], qss[:], AF.Sqrt)
        nc.gpsimd.tensor_scalar(qss[:], qss[:], 1e-12, None, ALU.max)
        qrs = self.t([128, 16, KNN], dt.float32, "qrs")
        nc.vector.reciprocal(qrs[:], qss[:])
        nc.gpsimd.tensor_tensor(quat[:], quat[:],
                                V(qrs, 0, [[KNN, 16], [1, KNN], [0, 4]]), ALU.mult)
        nc.sync.dma_start(DV(io['ange'], c * L * KNN * 4,
                             [[KNN * 4, 128], [128 * KNN * 4, 16], [1, KNN * 4]]),
                          V(quat, 0, [[KNN * 4, 16], [1, KNN * 4]]))

        # ---- direct_edge ----
        dire = self.bt([128, 16, KNN, 9], dt.float32, "dire", bufs=1)
        for ai, off in enumerate((A_N, A_C, A_O)):
            dv = self.bt([128, 16, KNN, 3], dt.float32, "edir", bufs=1)
            for ri in range(3):
                m = self.bt([128, 16, KNN, 3], dt.float32, "rm", bufs=1)
                nc.gpsimd.tensor_tensor(m[:], rq(3 + 3 * ri), self.ediffs[off][:],
                                        ALU.mult)
                nc.vector.tensor_reduce(V(dv, ri, [[KNN * 3, 16], [3, KNN], [1, 1]]),
                                        m[:], AX.X, ALU.add)
            ds = self.bt([128, 16, KNN, 3], dt.float32, "eds", bufs=1)
            nc.scalar.activation(ds[:], dv[:], AF.Square)
            dss = self.t([128, 16, KNN], dt.float32, "edss")
            nc.vector.tensor_reduce(dss[:], ds[:], AX.X, ALU.add)
            nc.scalar.activation(dss[:], dss[:], AF.Sqrt)
            nc.gpsimd.tensor_scalar(dss[:], dss[:], 1e-12, None, ALU.max)
            drs = self.t([128, 16, KNN], dt.float32, "edrs")
            nc.vector.reciprocal(drs[:], dss[:])
            nc.gpsimd.tensor_tensor(V(dire, ai * 3, [[KNN * 9, 16], [9, KNN], [1, 3]]),
                                    dv[:], V(drs, 0, [[KNN, 16], [1, KNN], [0, 3]]),
                                    ALU.mult)
        nc.sync.dma_start(DV(io['dire'], c * L * KNN * 9,
                             [[KNN * 9, 128], [128 * KNN * 9, 16], [1, KNN * 9]]),
                          V(dire, 0, [[KNN * 9, 16], [1, KNN * 9]]))

        # ---- masks ----
        ctxf = self.t([128, E], dt.float32, "ctxf")
        nc.vector.tensor_tensor(V(ctxf, 0, [[KNN, 16], [1, KNN]]),
                                x0b(A_SEG, 1), rq(12, 1), ALU.is_equal)
        ctx8 = self.t([128, E], dt.uint8, "ctx8")
        nc.vector.tensor_copy(ctx8[:], ctxf[:])
        nc.gpsimd.tensor_scalar(ctxf[:], ctxf[:], -1.0, 1.0, ALU.mult, ALU.add)
        it8 = self.t([128, E], dt.uint8, "it8")
        nc.vector.tensor_copy(it8[:], ctxf[:])
        nc.sync.dma_start(DV(io['ctx'], c * L * KNN,
                             [[KNN, 128], [128 * KNN, 16], [1, KNN]]),
                          V(ctx8, 0, [[KNN, 16], [1, KNN]]))
        nc.sync.dma_start(DV(io['intr'], c * L * KNN,
                             [[KNN, 128], [128 * KNN, 16], [1, KNN]]),
                          V(it8, 0, [[KNN, 16], [1, KNN]]))

        # ---- edge_pos (cos/sin via range-reduced Sin) ----
        dsh = self.t([128, E], dt.uint32, "dsh")
        io9 = self.t([128, 16, KNN], dt.uint32, "io9")
        nc.gpsimd.iota(io9[:], pattern=[[128, 16], [0, KNN]], base=2047,
                       channel_multiplier=1)
        nc.gpsimd.tensor_tensor(V(dsh, 0, [[KNN, 16], [1, KNN]]), io9[:],
                                V(self.col, 0, [[KNN, 16], [1, KNN]]), ALU.subtract)
        epos = self.ht([128, 16, KNN, 16], dt.float32, "epos")
        self.gather_by(
            lambda o, w: V(dsh, o, [[1, w]]),
            lambda o, w: V(epos, o * 16, [[16, w], [1, 16]]),
            DV(io['postable'], 0, [[64, 4095], [1, 16]]),
            E, 16, 64, "wrp")
        nc.sync.dma_start(DV(io['epos'], c * L * KNN * 16,
                             [[KNN * 16, 128], [128 * KNN * 16, 16], [1, KNN * 16]]),
                          V(epos, 0, [[KNN * 16, 16], [1, KNN * 16]]))


_BUILD_CACHE = {}


def _build():
    if 'nc' in _BUILD_CACHE:
        return _BUILD_CACHE['nc'], _BUILD_CACHE['io']
    nc = bacc.Bacc("TRN2", target_bir_lowering=False, debug=True,
                   num_devices=NCORE)
    for val in (EPS, 0.25e-12):
        tns = nc.alloc_sbuf_tensor(f"constap-{val}", [128, 1], dt.float32)
        nc.gpsimd.memset(tns.ap(), val)
        nc.const_aps.aps[(dt.float32, val)] = tns.ap()
    nc.all_engine_barrier()
    io = {}
    io['xplanes'] = nc.dram_tensor("xplanes", [CH, 4, L], dt.float32,
                                   kind="ExternalInput").ap()
    io['xrows'] = nc.dram_tensor("xrows", [CH, L, 16], dt.float32,
                                 kind="ExternalInput").ap()
    io['catab'] = nc.dram_tensor("catab", [CH, L, 64], dt.float32,
                                 kind="ExternalInput").ap()
    io['murow'] = nc.dram_tensor("murow", [128, 16], dt.float32,
                                 kind="ExternalInput").ap()
    io['postable'] = nc.dram_tensor("postable", [4095, 64], dt.float32,
                                    kind="ExternalInput").ap()
    io['angmul'] = nc.dram_tensor("angmul", [CH, 128, 192], dt.float32,
                                  kind="ExternalInput").ap()
    io['angadd'] = nc.dram_tensor("angadd", [CH, 128, 192], dt.float32,
                                  kind="ExternalInput").ap()
    io['framemask'] = nc.dram_tensor("framemask", [128, 16], dt.float32,
                                     kind="ExternalInput").ap()
    io['rectab'] = nc.dram_tensor("rectab", [CH, L, 64], dt.float32).ap()
    for name, shape, d in (
        ('col9', [CH, L, KNN], dt.uint32),
        ('disn', [CH, L, 48], dt.float32),
        ('angn', [CH, L, 12], dt.float32),
        ('dirn', [CH, L, 9], dt.float32),
        ('ctx', [CH, L, KNN], dt.uint8),
        ('intr', [CH, L, KNN], dt.uint8),
        ('epos', [CH, L * KNN, 16], dt.float32),
        ('dise', [CH, 4, L * KNN, 16], dt.float32),
        ('ange', [CH, L * KNN, 4], dt.float32),
        ('dire', [CH, L * KNN, 9], dt.float32),
    ):
        io[name] = nc.dram_tensor(name, shape, d, kind="ExternalOutput").ap()

    with tile.TileContext(nc) as tc:
        with tc.tile_pool(name="sb", bufs=2) as sb, \
             tc.tile_pool(name="big", bufs=2) as big, \
             tc.tile_pool(name="huge", bufs=1) as huge, \
             tc.tile_pool(name="ps", bufs=2, space="PSUM") as ps:
            pools = {'sb': sb, 'big': big, 'huge': huge, 'ps': ps}
            gsem = nc.alloc_semaphore()
            gcnt = [0]
            import os as _os
            nph = int(_os.environ.get("NPHASE", "5"))
            for c in range(CH):
                ch = _Chain(nc, tc, pools, io, c, gsem, gcnt)
                ch.setup()
                ch.node_feats()
                if nph >= 2:
                    ch.topk()
                if nph >= 3:
                    ch.refine()
                if nph >= 4:
                    ch.col_extract()
                if nph >= 5:
                    ch.edge_feats()
    nc.compile()
    _BUILD_CACHE['nc'] = nc
    _BUILD_CACHE['io'] = io
    return nc, io


def _pos_embed_np(pos):
    ang = pos[..., None].astype(np.float32) * POS_FREQ
    return np.concatenate([np.cos(ang), np.sin(ang)], -1).astype(np.float32)


def _make_in_maps(X, seg):
    murow = np.broadcast_to(MUS[None, :], (128, 16)).astype(np.float32).copy()
    framemask = np.ones((128, 16), np.float32)
    for n in (0, 1, 2046, 2047):
        framemask[n % 128, n // 128] = 0.0
    postable = np.zeros((4095, 64), np.float32)
    postable[:, 0:16] = _pos_embed_np(np.arange(-2047, 2048, dtype=np.float32))
    B = 16
    in_maps = []
    for g in range(NCORE):
        xplanes = np.zeros((CH, 4, L), np.float32)
        xrows = np.zeros((CH, L, 16), np.float32)
        catab = np.zeros((CH, L, 64), np.float32)
        angmul = np.ones((CH, 128, 16, 12), np.float32)
        angadd = np.zeros((CH, 128, 16, 12), np.float32)
        for c in range(CH):
            chain = 2 * g + c
            Xc = X[chain * L:(chain + 1) * L]
            sc = seg[chain * L:(chain + 1) * L]
            xplanes[c, 0:3] = Xc[:, 1, :].T
            xplanes[c, 3] = 1.0
            xrows[c, :, 0:12] = Xc.reshape(L, 12)
            xrows[c, :, 12] = sc.astype(np.float32)
            catab[c, :, 0:3] = Xc[:, 1, :]
            for n in (0, 1):
                angmul[c, n % 128, n // 128, :] = 0.0
            n = L - 1
            if chain != B - 1:
                angmul[c, n % 128, n // 128, :] = 0.0
            else:
                angmul[c, n % 128, n // 128, :] = np.array(
                    [1, 0, 0, 1, 0, 0, 1, 0, 0, 1, 0, 0], np.float32)
                angadd[c, n % 128, n // 128, :] = np.array(
                    [0, 1, 1, 0, 0, 0, 0, 1, 1, 0, 0, 0], np.float32)
        in_maps.append({
            "xplanes": xplanes, "xrows": xrows, "catab": catab,
            "murow": murow, "framemask": framemask, "postable": postable,
            "angmul": angmul.reshape(CH, 128, 192),
            "angadd": angadd.reshape(CH, 128, 192),
        })
    return in_maps


def _assemble(r, N):
    col = np.concatenate([r[g]['col9'].reshape(CH * L, KNN) for g in range(NCORE)], 0)
    chain_of = np.arange(N) // L
    col_g = (col.astype(np.int64) + (chain_of * L)[:, None]).astype(np.int32)
    row = np.broadcast_to(np.arange(N, dtype=np.int32)[:, None], (N, KNN))
    edge_index = np.stack([row.reshape(-1).copy(), col_g.reshape(-1)]).astype(np.int32)
    node_pos = np.tile(_pos_embed_np(np.arange(L, dtype=np.float32)), (16, 1))
    cat = lambda k, w: np.concatenate(
        [r[g][k].reshape(-1, w) if w else r[g][k].reshape(-1)
         for g in range(NCORE)], 0)
    dise = np.concatenate(
        [r[g]['dise'].reshape(CH, 4, L * KNN, 16).transpose(0, 2, 1, 3)
         .reshape(CH * L * KNN, 64) for g in range(NCORE)], 0)
    return (node_pos, cat('disn', 48), cat('angn', 12), cat('dirn', 9), edge_index,
            cat('ctx', 0) != 0, cat('intr', 0) != 0, cat('epos', 16),
            dise, cat('ange', 4), cat('dire', 9))


def kernel(X, segment_ids):
    X = np.asarray(X, dtype=np.float32)
    seg = np.asarray(segment_ids, dtype=np.int32)
    N = X.shape[0]
    assert N == 16 * L
    nc, _ = _build()
    in_maps = _make_in_maps(X, seg)
    res = run_bass_kernel_spmd(nc, in_maps, core_ids=list(range(NCORE)))
    return _assemble(res.results, N)


# revision 22
# speedup vs baseline: 90.4125x; 90.4125x over previous
"""Trainium2 Bass kernel for protein-feature GNN message passing.

Sharding: data-parallel over 16 chains -> 8 cores x 2 chains.
Per chain (L=2048): KNN top-9 via fp32r matmul (2a.b - |b|^2, monotone in -d^2
per row) -> per-quarter top-8 candidates (DVE max8/max_index) -> exact fp32
re-rank of 32 candidates -> node/edge features on ACT/GPSIMD -> DMA out.
"""
import sys
sys.path.insert(0, '/opt/trn_rl_repo')
import numpy as np

import inspect as _inspect
import concourse.bass as bass
import concourse.bacc as bacc
import concourse.mybir as mybir
from concourse import tile
from concourse.bass import AP
from concourse.bass_utils import run_bass_kernel_spmd

# dma_gather's 256B elem assert is a transpose-mode restriction; small payloads
# with 256B row STRIDE work (HW-verified). Patch the assert.
_gsrc = _inspect.getsource(bass.BassGpSimd.dma_gather)
_gsrc = _gsrc.replace("elem_size_bytes > 0 and elem_size_bytes % 256 == 0",
                      "elem_size_bytes > 0")
_gns = dict(bass.__dict__)
exec("def _patched" + _gsrc[len("    def dma_gather"):].replace("\n    ", "\n"), _gns)
bass.BassGpSimd.dma_gather = _gns["_patched"]

dt = mybir.dt
AF = mybir.ActivationFunctionType
ALU = mybir.AluOpType
AX = mybir.AxisListType

L = 2048
NBLK = 16
CH = 2
NCORE = 8
KNN = 9
NCAND = 32
E = 16 * KNN              # 144 edge slots per partition
EPS = 1e-6
SIGMA = 20.0 / 16.0
TWO_PI = float(2.0 * np.pi)
HALF_PI = float(np.pi / 2.0)
PI = float(np.pi)
MUS = np.linspace(0.0, 20.0, 16).astype(np.float32)
POS_FREQ = np.exp(np.arange(0, 16, 2, dtype=np.float32) *
                  (-np.log(10000.0) / 16)).astype(np.float32)

A_N, A_CA, A_C, A_O, A_SEG = 0, 3, 6, 9, 12


def V(t, off, dims):
    """Free-dim view on an SBUF tile (keeps partition dim)."""
    return AP(t.tensor, t.offset + off, [list(t.ap[0])] + [list(d) for d in dims])


def PV(t, poff, dims):
    """View with partition offset (for the 16-wide wrap tiles)."""
    ap0 = list(t.ap[0])
    return AP(t.tensor, t.offset + poff, [ap0] + [list(d) for d in dims])


def DV(d, off, dims):
    return AP(d.tensor, d.offset + off, [list(x) for x in dims])


class _Chain:
    def __init__(self, nc, tc, pools, io, c, gsem, gcnt):
        self.nc = nc
        self.tc = tc
        self.gsem = gsem
        self.gcnt = gcnt
        self.sb = pools['sb']
        self.big = pools['big']
        self.huge = pools['huge']
        self.ps = pools['ps']
        self.io = io
        self.c = c

    def t(self, shape, dtype, tag, bufs=None):
        return self.sb.tile(shape, dtype, tag=tag, name=tag or "tmp", bufs=bufs)

    def bt(self, shape, dtype, tag, bufs=None):
        return self.big.tile(shape, dtype, tag=tag, name=tag or "tmp", bufs=bufs)

    def ht(self, shape, dtype, tag):
        return self.huge.tile(shape, dtype, tag=tag, name=tag or "tmp")

    # ---------- helpers ----------
    def norm3(self, u, tag, out_ap=None):
        """normalize [128,16,3]: u * rsqrt(max(sumsq, 1e-24))."""
        nc = self.nc
        squ = self.t([128, 16, 3], dt.float32, "h_sq")
        nc.scalar.activation(squ[:], u[:], AF.Square)
        ss = self.t([128, 16], dt.float32, "h_ss")
        nc.vector.tensor_reduce(ss[:], squ[:], AX.X, ALU.add)
        nr = self.t([128, 16], dt.float32, "h_nr")
        nc.scalar.activation(nr[:], ss[:], AF.Sqrt)
        nc.gpsimd.tensor_scalar(nr[:], nr[:], 1e-12, None, ALU.max)
        rs = self.t([128, 16], dt.float32, "h_rs")
        nc.vector.reciprocal(rs[:], nr[:])
        if out_ap is None:
            un = self.t([128, 16, 3], dt.float32, tag)
            out_ap = un[:]
        else:
            un = None
        nc.gpsimd.tensor_tensor(out_ap, u[:], V(rs, 0, [[1, 16], [0, 3]]), ALU.mult)
        return un

    def rot(self, u, which, tag):
        nc = self.nc
        r = self.t([128, 16, 3], dt.float32, tag)
        if which == 1:      # (y, z, x)
            nc.scalar.copy(V(r, 0, [[3, 16], [1, 2]]), V(u, 1, [[3, 16], [1, 2]]))
            nc.scalar.copy(V(r, 2, [[3, 16], [1, 1]]), V(u, 0, [[3, 16], [1, 1]]))
        else:               # (z, x, y)
            nc.scalar.copy(V(r, 0, [[3, 16], [1, 1]]), V(u, 2, [[3, 16], [1, 1]]))
            nc.scalar.copy(V(r, 1, [[3, 16], [1, 2]]), V(u, 0, [[3, 16], [1, 2]]))
        return r

    def cross(self, tag, a1, a2, b1, b2):
        nc = self.nc
        m1 = self.t([128, 16, 3], dt.float32, "h_m1")
        nc.gpsimd.tensor_tensor(m1[:], a1[:], b2[:], ALU.mult)
        m2 = self.t([128, 16, 3], dt.float32, "h_m2")
        nc.gpsimd.tensor_tensor(m2[:], a2[:], b1[:], ALU.mult)
        w = self.t([128, 16, 3], dt.float32, tag)
        nc.gpsimd.tensor_tensor(w[:], m1[:], m2[:], ALU.subtract)
        return w

    def dot16(self, a_ap, b_ap, out_ap):
        nc = self.nc
        m = self.t([128, 16, 3], dt.float32, "h_dm")
        nc.gpsimd.tensor_tensor(m[:], a_ap, b_ap, ALU.mult)
        nc.vector.tensor_reduce(out_ap, m[:], AX.X, ALU.add)

    def wrap_idx(self, src_u32_ap, width, tag):
        """[128, width] u32 AP -> replicated wrapped int16 idx tile [128, 8*width]."""
        nc = self.nc
        wr32 = self.t([16, 8 * 8 * NCAND], dt.uint32, "wrap32", bufs=1)
        for ph in range(8):
            nc.sync.dma_start(PV(wr32, ph, [[8, width]]),
                              AP(src_u32_ap.tensor,
                                 src_u32_ap.offset + ph * 16 * src_u32_ap.ap[0][0],
                                 [[src_u32_ap.ap[0][0], 16]] + src_u32_ap.ap[1:]))
        wr = self.t([16, 8 * 8 * NCAND], dt.int16, "wrap16", bufs=1)
        nc.gpsimd.tensor_copy(wr[:, :8 * width], wr32[:, :8 * width])
        wrf = self.t([128, 8 * 8 * NCAND], dt.int16, "wrapf", bufs=2)
        for g in range(8):
            nc.sync.dma_start(wrf[g * 16:(g + 1) * 16, :8 * width],
                              wr[:, :8 * width])
        return wrf

    def gather(self, out_ap, in_ap, wrf, nidx, elem_size, elem_step):
        nc = self.nc
        with self.tc.tile_critical():
            nc.gpsimd.dma_gather(out_ap, in_ap, wrf[:], nidx, nidx, elem_size,
                                 elem_step=elem_step,
                                 single_packet=False).then_inc(self.gsem, 16)
            self.gcnt[0] += 16
            nc.gpsimd.wait_ge(self.gsem, self.gcnt[0])

    def gather_by(self, src_tile_ap_fn, out_fn, in_ap, total_w, elem_size,
                  elem_step, tag, chunk=64):
        """Chunked gather: per chunk of width<=64 idx columns, wrap + gather."""
        off = 0
        while off < total_w:
            w = min(chunk, total_w - off)
            wrf = self.wrap_idx(src_tile_ap_fn(off, w), w, tag)
            self.gather(out_fn(off, w), in_ap, wrf, w * 128, elem_size, elem_step)
            off += w

    # ---------- phases ----------
    def setup(self):
        nc, io, c = self.nc, self.io, self.c
        self.xp = self.t([4, L], dt.float32, "ptmp", bufs=1)
        nc.sync.dma_start(self.xp[:], DV(io['xplanes'], c * 4 * L, [[L, 4], [1, L]]))
        self.xpr = self.t([4, L], dt.float32r, "xpr", bufs=1)
        nc.scalar.activation(self.xpr[:], self.xp[:], AF.Copy)
        sq = self.t([3, L], dt.float32r, "ptmp", bufs=1)
        nc.scalar.activation(sq[:], self.xpr[0:3, :].bitcast(dt.float32), AF.Square)
        ones31 = self.t([3, 1], dt.float32r, "ones31")
        nc.gpsimd.memset(ones31[:].bitcast(dt.float32), -1.0)
        rr = self.ps.tile([128, L], dt.float32, tag="dmat", name="rr")
        for q in range(4):
            nc.tensor.matmul(rr[0:1, q * 512:(q + 1) * 512], ones31[:],
                             sq[:, q * 512:(q + 1) * 512])
        self.v4 = self.t([4, L], dt.float32r, "v4", bufs=1)
        nc.scalar.activation(self.v4[0:3, :], self.xpr[0:3, :].bitcast(dt.float32),
                             AF.Copy, scale=2.0)
        rtmp = self.t([1, L], dt.float32, "rtmp", bufs=1)
        nc.scalar.activation(rtmp[:], rr[0:1, :], AF.Copy)
        nc.sync.dma_start(self.v4[3:4, :].bitcast(dt.float32), rtmp[:])

        base = c * L * 16
        self.xr0 = self.t([128, 16, 16], dt.float32, "xr0")
        nc.sync.dma_start(self.xr0[:],
                          DV(io['xrows'], base, [[16, 128], [128 * 16, 16], [1, 16]]))
        self.xrm = self.t([128, 16, 16], dt.float32, "xrm")
        nc.gpsimd.memset(self.xrm[:], 0.0)
        nc.sync.dma_start(self.xrm[1:128, 0:1, :],
                          DV(io['xrows'], base, [[16, 127], [1, 16]]))
        nc.sync.dma_start(self.xrm[:, 1:16, :],
                          DV(io['xrows'], base + 127 * 16,
                             [[16, 128], [128 * 16, 15], [1, 16]]))
        self.xrp = self.t([128, 16, 16], dt.float32, "xrp")
        nc.gpsimd.memset(self.xrp[:], 0.0)
        nc.sync.dma_start(self.xrp[:, 0:15, :],
                          DV(io['xrows'], base + 16,
                             [[16, 128], [128 * 16, 15], [1, 16]]))
        nc.sync.dma_start(self.xrp[0:127, 15:16, :],
                          DV(io['xrows'], base + 1921 * 16, [[16, 127], [1, 16]]))
        self.mu = self.t([128, 16], dt.float32, "mu")
        nc.sync.dma_start(self.mu[:], io['murow'][:])

    def topk(self):
        nc = self.nc
        self.cidx = self.t([128, 16, NCAND], dt.uint32, "cidx")
        for b in range(NBLK):
            dmat = self.ps.tile([128, L], dt.float32, tag="dmat", name="dmat")
            for q in range(4):
                nc.tensor.matmul(dmat[:, q * 512:(q + 1) * 512],
                                 self.xpr[:, b * 128:(b + 1) * 128],
                                 self.v4[:, q * 512:(q + 1) * 512])
            for q in range(4):
                vq = self.t([128, 8], dt.float32, "vq")
                win = dmat[:, q * 512:(q + 1) * 512]
                nc.vector.max(vq[:], win)
                nc.vector.max_index(V(self.cidx, b * NCAND + q * 8, [[1, 8]]),
                                    vq[:], win)

    def refine(self):
        import os as _os
        refk = int(_os.environ.get("REFK", "9"))
        nc, io, c = self.nc, self.io, self.c
        qoff = self.t([128, 16, NCAND], dt.uint32, "qoff", bufs=1)
        nc.gpsimd.iota(V(qoff, 0, [[NCAND, 16], [8, 4], [1, 8]]),
                       pattern=[[0, 16], [512, 4], [0, 8]], base=0,
                       channel_multiplier=0)
        nc.gpsimd.tensor_tensor(self.cidx[:], self.cidx[:], qoff[:], ALU.add)
        self.cg = self.cidx
        ccand = self.bt([128, 16 * NCAND, 3], dt.float32, "ccand", bufs=1)
        if refk < 2:
            nc.gpsimd.memset(ccand[:], 1.0)
        else:
            self.gather_by(
                lambda o, w: V(self.cg, o, [[1, w]]),
                lambda o, w: V(ccand, o * 3, [[3, w], [1, 3]]),
                DV(io['catab'], c * L * 64, [[64, L], [1, 3]]),
                16 * NCAND, 3, 64, "wrc")
        if refk < 3:
            nc.gpsimd.memset(V(ccand, 0, [[1, 100]]), 0.5)
        import os as _os3
        if _os3.environ.get("DBGCG", "0") == "1":
            nc.sync.dma_start(DV(io['col9'], c * L * KNN,
                                 [[KNN, 128], [128 * KNN, 16], [1, KNN]]),
                              V(self.cg, 0, [[NCAND, 16], [1, KNN]]))
        dview = V(ccand, 0, [[NCAND * 3, 16], [3, NCAND], [1, 3]])
        nc.gpsimd.tensor_tensor(dview, dview,
                                V(self.xr0, A_CA, [[16, 16], [0, NCAND], [1, 3]]),
                                ALU.subtract)
        nc.scalar.activation(dview, dview, AF.Square)
        self.negd2 = self.t([128, 16, NCAND], dt.float32, "negd2")
        nc.vector.tensor_reduce(self.negd2[:], dview, AX.X, ALU.add)
        nc.gpsimd.tensor_scalar(self.negd2[:], self.negd2[:], -1.0, None, ALU.mult)
        self.pos = self.t([128, 16, 16], dt.uint32, "pos")
        for b in range(NBLK):
            nv = V(self.negd2, b * NCAND, [[1, NCAND]])
            v8 = self.t([128, 8], dt.float32, "rv8")
            nc.vector.max(v8[:], nv)
            mr = self.t([128, NCAND], dt.float32, "rmr")
            nc.vector.match_replace(mr[:], v8[:], nv, -1e30)
            v9 = self.t([128, 8], dt.float32, "rv9")
            nc.vector.max(v9[:], mr[:])
            nc.vector.max_index(V(self.pos, b * 16, [[1, 8]]), v8[:], nv)
            nc.vector.max_index(V(self.pos, b * 16 + 8, [[1, 8]]), v9[:], mr[:])

    def col_extract(self):
        """col[p,b,k] = cg[p,b,pos[p,b,k]] via one-hot eq-select."""
        nc, io, c = self.nc, self.io, self.c
        iotac = self.t([128, NCAND], dt.uint32, "iotac")
        nc.gpsimd.iota(iotac[:], pattern=[[1, NCAND]], base=0, channel_multiplier=0)
        cgf = self.t([128, 16, NCAND], dt.float32, "cgf")
        nc.vector.tensor_copy(cgf[:], self.cg[:])
        colf = self.t([128, 16, KNN], dt.float32, "colf")
        HC = NCAND // 2
        for hf in range(2):
            mask = self.bt([128, 16, KNN, HC], dt.float32, "emask", bufs=1)
            nc.vector.tensor_tensor(mask[:],
                                    V(iotac, hf * HC, [[0, 16], [0, KNN], [1, HC]]),
                                    V(self.pos, 0, [[16, 16], [1, KNN], [0, HC]]),
                                    ALU.is_equal)
            nc.vector.tensor_tensor(mask[:], mask[:],
                                    V(cgf, hf * HC, [[NCAND, 16], [0, KNN], [1, HC]]),
                                    ALU.mult)
            if hf == 0:
                nc.vector.tensor_reduce(colf[:], mask[:], AX.X, ALU.add)
            else:
                half2 = self.t([128, 16, KNN], dt.float32, "colf2")
                nc.vector.tensor_reduce(half2[:], mask[:], AX.X, ALU.add)
                nc.gpsimd.tensor_tensor(colf[:], colf[:], half2[:], ALU.add)
        self.col = self.t([128, E], dt.uint32, "col")
        nc.gpsimd.tensor_copy(V(self.col, 0, [[KNN, 16], [1, KNN]]), colf[:])
        nc.sync.dma_start(DV(io['col9'], c * L * KNN,
                             [[KNN, 128], [128 * KNN, 16], [1, KNN]]),
                          V(self.col, 0, [[KNN, 16], [1, KNN]]))

    def node_feats(self):
        nc, io, c = self.nc, self.io, self.c
        x0, xm, xp_ = self.xr0, self.xrm, self.xrp

        def sl(t, off):
            return V(t, off, [[16, 16], [1, 3]])

        # ---- dis_node ----
        disn = self.bt([128, 16, 48], dt.float32, "disn", bufs=1)
        self.ndiff = {}
        for pi, off in enumerate((A_N, A_C, A_O)):
            d3 = self.t([128, 16, 3], dt.float32, f"nd{pi}")
            nc.gpsimd.tensor_tensor(d3[:], sl(x0, off), sl(x0, A_CA), ALU.subtract)
            self.ndiff[off] = d3
            s3 = self.t([128, 16, 3], dt.float32, "h_sq")
            nc.scalar.activation(s3[:], d3[:], AF.Square)
            d2 = self.t([128, 16], dt.float32, "h_ss")
            nc.vector.tensor_reduce(d2[:], s3[:], AX.X, ALU.add)
            dd = self.t([128, 16], dt.float32, "ndd")
            nc.scalar.activation(dd[:], d2[:], AF.Sqrt, bias=EPS)
            um = self.t([128, 16, 16], dt.float32, "num")
            nc.gpsimd.tensor_tensor(um[:], V(dd, 0, [[1, 16], [0, 16]]),
                                    V(self.mu, 0, [[0, 16], [1, 16]]), ALU.subtract)
            nc.scalar.activation(um[:], um[:], AF.Square, scale=1.0 / SIGMA)
            nc.scalar.activation(V(disn, pi * 16, [[48, 16], [1, 16]]), um[:],
                                 AF.Exp, scale=-1.0)
        nc.sync.dma_start(DV(io['disn'], c * L * 48,
                             [[48, 128], [128 * 48, 16], [1, 48]]),
                          V(disn, 0, [[48, 16], [1, 48]]))

        # ---- angles ----
        ua = self.t([128, 16, 3], dt.float32, "ua")
        nc.gpsimd.tensor_tensor(ua[:], sl(x0, A_N), sl(xm, A_C), ALU.subtract)
        ub = self.t([128, 16, 3], dt.float32, "ub")
        nc.gpsimd.tensor_tensor(ub[:], sl(x0, A_CA), sl(x0, A_N), ALU.subtract)
        uc = self.t([128, 16, 3], dt.float32, "uc")
        nc.gpsimd.tensor_tensor(uc[:], sl(x0, A_C), sl(x0, A_CA), ALU.subtract)
        ud = self.t([128, 16, 3], dt.float32, "ud")
        nc.gpsimd.tensor_tensor(ud[:], sl(xp_, A_N), sl(x0, A_C), ALU.subtract)
        ue = self.t([128, 16, 3], dt.float32, "ue")
        nc.gpsimd.tensor_tensor(ue[:], sl(xp_, A_CA), sl(xp_, A_N), ALU.subtract)
        us = [self.norm3(u, f"un{i}") for i, u in enumerate((ua, ub, uc, ud, ue))]
        rots = {}
        for i, u in enumerate(us):
            rots[(i, 1)] = self.rot(u, 1, f"ur{i}1")
            rots[(i, 2)] = self.rot(u, 2, f"ur{i}2")
        ws = []
        for i in range(4):
            w = self.cross(f"wx{i}", rots[(i, 1)], rots[(i, 2)],
                           rots[(i + 1, 1)], rots[(i + 1, 2)])
            ws.append(self.norm3(w, f"wn{i}"))
        ang = self.bt([128, 16, 12], dt.float32, "angn")
        cosd = self.t([128, 16, 3], dt.float32, "cosd")
        sgn = self.t([128, 16, 3], dt.float32, "sgn")
        cosa = self.t([128, 16, 3], dt.float32, "cosa")
        for ti in range(3):
            self.dot16(ws[ti][:], ws[ti + 1][:], V(cosd, ti, [[3, 16]]))
            self.dot16(us[ti][:], ws[ti + 1][:], V(sgn, ti, [[3, 16]]))
            self.dot16(us[ti][:], us[ti + 1][:], V(cosa, ti, [[3, 16]]))
        nc.scalar.activation(sgn[:], sgn[:], AF.Sign)
        CL = 1.0 - EPS
        nc.gpsimd.tensor_scalar(V(ang, 0, [[12, 16], [1, 3]]), cosd[:], CL, -CL,
                                ALU.min, ALU.max)
        nc.gpsimd.tensor_scalar(V(ang, 6, [[12, 16], [1, 3]]), cosa[:], CL, -CL,
                                ALU.min, ALU.max)
        for bse, sg in ((0, sgn), (6, None)):
            s2 = self.t([128, 16, 3], dt.float32, "s2")
            nc.scalar.activation(s2[:], V(ang, bse, [[12, 16], [1, 3]]), AF.Square)
            nc.gpsimd.tensor_scalar(s2[:], s2[:], -1.0, 1.0, ALU.mult, ALU.add)
            sr = self.t([128, 16, 3], dt.float32, "sr")
            nc.scalar.activation(sr[:], s2[:], AF.Sqrt)
            if sg is not None:
                nc.gpsimd.tensor_tensor(V(ang, bse + 3, [[12, 16], [1, 3]]), sr[:],
                                        sg[:], ALU.mult)
            else:
                nc.scalar.copy(V(ang, bse + 3, [[12, 16], [1, 3]]), sr[:])
        angm = self.t([128, 16, 12], dt.float32, "angm")
        nc.sync.dma_start(angm[:], DV(io['angmul'], c * 128 * 192,
                                      [[192, 128], [12, 16], [1, 12]]))
        anga = self.t([128, 16, 12], dt.float32, "anga")
        nc.sync.dma_start(anga[:], DV(io['angadd'], c * 128 * 192,
                                      [[192, 128], [12, 16], [1, 12]]))
        nc.gpsimd.tensor_tensor(ang[:], ang[:], angm[:], ALU.mult)
        nc.gpsimd.tensor_tensor(ang[:], ang[:], anga[:], ALU.add)
        nc.sync.dma_start(DV(io['angn'], c * L * 12,
                             [[12, 128], [128 * 12, 16], [1, 12]]),
                          V(ang, 0, [[12, 16], [1, 12]]))

        # ---- frames (into one oall [128,16,9] tile: o1,nf,o2) ----
        dA = self.t([128, 16, 3], dt.float32, "dA")
        nc.gpsimd.tensor_tensor(dA[:], sl(x0, A_CA), sl(xm, A_CA), ALU.subtract)
        ucA = self.norm3(dA, "ucA")
        dB = self.t([128, 16, 3], dt.float32, "dB")
        nc.gpsimd.tensor_tensor(dB[:], sl(xp_, A_CA), sl(x0, A_CA), ALU.subtract)
        ucB = self.norm3(dB, "ucB")
        nfr = self.cross("nfx", self.rot(ucA, 1, "ra1"), self.rot(ucA, 2, "ra2"),
                         self.rot(ucB, 1, "rb1"), self.rot(ucB, 2, "rb2"))
        nf = self.norm3(nfr, "nf")
        do1 = self.t([128, 16, 3], dt.float32, "do1")
        nc.gpsimd.tensor_tensor(do1[:], ucA[:], ucB[:], ALU.subtract)
        o1 = self.norm3(do1, "o1")
        o2 = self.cross("o2x", self.rot(o1, 1, "ro1"), self.rot(o1, 2, "ro2"),
                        self.rot(nf, 1, "rn1"), self.rot(nf, 2, "rn2"))
        fm = self.t([128, 16], dt.float32, "fm")
        nc.sync.dma_start(fm[:], io['framemask'][:])
        fmb = V(fm, 0, [[1, 16], [0, 3]])
        self.oall = self.bt([128, 16, 9], dt.float32, "oall")
        nc.gpsimd.tensor_tensor(V(self.oall, 0, [[9, 16], [1, 3]]), o1[:], fmb,
                                ALU.mult)
        nc.gpsimd.tensor_tensor(V(self.oall, 3, [[9, 16], [1, 3]]), nf[:], fmb,
                                ALU.mult)
        nc.gpsimd.tensor_tensor(V(self.oall, 6, [[9, 16], [1, 3]]), o2[:], fmb,
                                ALU.mult)
        rec = self.bt([128, 16, 16], dt.float32, "rec")
        nc.scalar.copy(V(rec, 0, [[16, 16], [1, 3]]), sl(x0, A_CA))
        nc.scalar.copy(V(rec, 3, [[16, 16], [1, 9]]), V(self.oall, 0, [[9, 16], [1, 9]]))
        nc.scalar.copy(V(rec, 12, [[16, 16], [1, 1]]), V(x0, A_SEG, [[16, 16], [1, 1]]))
        nc.gpsimd.memset(V(rec, 13, [[16, 16], [1, 3]]), 0.0)
        nc.sync.dma_start(DV(io['rectab'], c * L * 64,
                             [[64, 128], [128 * 64, 16], [1, 16]]),
                          V(rec, 0, [[16, 16], [1, 16]]))

        # ---- direct_node ----
        dirn = self.bt([128, 16, 9], dt.float32, "dirn")
        for ai, off in enumerate((A_N, A_C, A_O)):
            dv = self.t([128, 16, 3], dt.float32, "dv")
            for ri in range(3):
                self.dot16(V(self.oall, 3 * ri, [[9, 16], [1, 3]]),
                           self.ndiff[off][:], V(dv, ri, [[3, 16]]))
            self.norm3(dv, "", out_ap=V(dirn, ai * 3, [[9, 16], [1, 3]]))
        nc.sync.dma_start(DV(io['dirn'], c * L * 9,
                             [[9, 128], [128 * 9, 16], [1, 9]]),
                          V(dirn, 0, [[9, 16], [1, 9]]))

    def edge_feats(self):
        nc, io, c = self.nc, self.io, self.c
        recq = self.ht([128, E, 16], dt.float32, "recq")
        self.gather_by(
            lambda o, w: V(self.col, o, [[1, w]]),
            lambda o, w: V(recq, o * 16, [[16, w], [1, 16]]),
            DV(io['rectab'], c * L * 64, [[64, L], [1, 16]]),
            E, 16, 64, "wrr")

        def rq(off, n=3):
            return V(recq, off, [[16 * KNN, 16], [16, KNN], [1, n]])

        def x0b(off, n=3):
            return V(self.xr0, off, [[16, 16], [0, KNN], [1, n]])

        # ---- dis_edge (atom-major DRAM; host reorders columns) ----
        self.ediffs = {}
        for ai, off in enumerate((A_N, A_CA, A_C, A_O)):
            de3 = self.bt([128, 16, KNN, 3], dt.float32, f"ed{ai}", bufs=1)
            nc.vector.tensor_tensor(de3[:], x0b(off), rq(0), ALU.subtract)
            self.ediffs[off] = de3
            se3 = self.bt([128, 16, KNN, 3], dt.float32, "eds", bufs=1)
            nc.scalar.activation(se3[:], de3[:], AF.Square)
            e2 = self.t([128, 16, KNN], dt.float32, "ed2")
            nc.vector.tensor_reduce(e2[:], se3[:], AX.X, ALU.add)
            ed = self.t([128, 16, KNN], dt.float32, "edd")
            nc.scalar.activation(ed[:], e2[:], AF.Sqrt, bias=EPS)
            ue = self.ht([128, 16, KNN, 16], dt.float32, "eum")
            nc.vector.tensor_tensor(ue[:], V(ed, 0, [[KNN, 16], [1, KNN], [0, 16]]),
                                    V(self.mu, 0, [[0, 16], [0, KNN], [1, 16]]),
                                    ALU.subtract)
            nc.scalar.activation(ue[:], ue[:], AF.Square, scale=1.0 / SIGMA)
            nc.scalar.activation(ue[:], ue[:], AF.Exp, scale=-1.0)
            nc.sync.dma_start(
                DV(io['dise'], (c * 4 + ai) * L * KNN * 16,
                   [[KNN * 16, 128], [128 * KNN * 16, 16], [1, KNN * 16]]),
                V(ue, 0, [[KNN * 16, 16], [1, KNN * 16]]))

        # ---- angle_edge: R_ij = sum_k Or[k,i] * Oc[k,j] ----
        R = [[None] * 3 for _ in range(3)]
        for i in range(3):
            ob = V(self.oall, i, [[9, 16], [0, KNN], [3, 3]])
            for j in range(3):
                m = self.bt([128, 16, KNN, 3], dt.float32, "rm", bufs=1)
                nc.gpsimd.tensor_tensor(
                    m[:], ob, V(recq, 3 + j, [[16 * KNN, 16], [16, KNN], [3, 3]]),
                    ALU.mult)
                rij = self.t([128, 16, KNN], dt.float32, f"rr{i}{j}")
                nc.vector.tensor_reduce(rij[:], m[:], AX.X, ALU.add)
                R[i][j] = rij
        quat = self.bt([128, 16, KNN, 4], dt.float32, "quat", bufs=1)
        mags = self.bt([128, 16, KNN, 3], dt.float32, "qmags", bufs=1)
        for k in range(3):
            t2 = self.t([128, 16, KNN], dt.float32, "qt")
            if k == 0:
                nc.gpsimd.tensor_tensor(t2[:], R[0][0][:], R[1][1][:], ALU.subtract)
                nc.gpsimd.tensor_tensor(t2[:], t2[:], R[2][2][:], ALU.subtract)
            elif k == 1:
                nc.gpsimd.tensor_tensor(t2[:], R[1][1][:], R[0][0][:], ALU.subtract)
                nc.gpsimd.tensor_tensor(t2[:], t2[:], R[2][2][:], ALU.subtract)
            else:
                nc.gpsimd.tensor_tensor(t2[:], R[2][2][:], R[0][0][:], ALU.subtract)
                nc.gpsimd.tensor_tensor(t2[:], t2[:], R[1][1][:], ALU.subtract)
            nc.scalar.activation(t2[:], t2[:], AF.Abs, bias=1.0)
            nc.scalar.activation(V(mags, k, [[KNN * 3, 16], [3, KNN], [1, 1]]),
                                 t2[:], AF.Sqrt, scale=0.25, bias=0.25e-12)
        sgns = self.bt([128, 16, KNN, 3], dt.float32, "qsgn", bufs=1)
        for k, (i, j) in enumerate(((2, 1), (0, 2), (1, 0))):
            nc.gpsimd.tensor_tensor(V(sgns, k, [[KNN * 3, 16], [3, KNN], [1, 1]]),
                                    R[i][j][:], R[j][i][:], ALU.subtract)
        nc.scalar.activation(sgns[:], sgns[:], AF.Sign)
        nc.gpsimd.tensor_tensor(V(quat, 0, [[KNN * 4, 16], [4, KNN], [1, 3]]),
                                sgns[:], mags[:], ALU.mult)
        tr = self.t([128, 16, KNN], dt.float32, "qtr")
        nc.gpsimd.tensor_tensor(tr[:], R[0][0][:], R[1][1][:], ALU.add)
        nc.gpsimd.tensor_tensor(tr[:], tr[:], R[2][2][:], ALU.add)
        nc.scalar.activation(tr[:], tr[:], AF.Relu, bias=1.0)
        nc.scalar.activation(V(quat, 3, [[KNN * 4, 16], [4, KNN], [1, 1]]), tr[:],
                             AF.Sqrt, scale=0.25, bias=0.25e-12)
        qs = self.bt([128, 16, KNN, 4], dt.float32, "qsq", bufs=1)
        nc.scalar.activation(qs[:], quat[:], AF.Square)
        qss = self.t([128, 16, KNN], dt.float32, "qss")
        nc.vector.tensor_reduce(qss[:], qs[:], AX.X, ALU.add)
        nc.scalar.activation(qss[:], qss[:], AF.Sqrt)
        nc.gpsimd.tensor_scalar(qss[:], qss[:], 1e-12, None, ALU.max)
        qrs = self.t([128, 16, KNN], dt.float32, "qrs")
        nc.vector.reciprocal(qrs[:], qss[:])
        nc.gpsimd.tensor_tensor(quat[:], quat[:],
                                V(qrs, 0, [[KNN, 16], [1, KNN], [0, 4]]), ALU.mult)
        nc.sync.dma_start(DV(io['ange'], c * L * KNN * 4,
                             [[KNN * 4, 128], [128 * KNN * 4, 16], [1, KNN * 4]]),
                          V(quat, 0, [[KNN * 4, 16], [1, KNN * 4]]))

        # ---- direct_edge ----
        dire = self.bt([128, 16, KNN, 9], dt.float32, "dire", bufs=1)
        for ai, off in enumerate((A_N, A_C, A_O)):
            dv = self.bt([128, 16, KNN, 3], dt.float32, "edir", bufs=1)
            for ri in range(3):
                m = self.bt([128, 16, KNN, 3], dt.float32, "rm", bufs=1)
                nc.gpsimd.tensor_tensor(m[:], rq(3 + 3 * ri), self.ediffs[off][:],
                                        ALU.mult)
                nc.vector.tensor_reduce(V(dv, ri, [[KNN * 3, 16], [3, KNN], [1, 1]]),
                                        m[:], AX.X, ALU.add)
            ds = self.bt([128, 16, KNN, 3], dt.float32, "eds", bufs=1)
            nc.scalar.activation(ds[:], dv[:], AF.Square)
            dss = self.t([128, 16, KNN], dt.float32, "edss")
            nc.vector.tensor_reduce(dss[:], ds[:], AX.X, ALU.add)
            nc.scalar.activation(dss[:], dss[:], AF.Sqrt)
            nc.gpsimd.tensor_scalar(dss[:], dss[:], 1e-12, None, ALU.max)
            drs = self.t([128, 16, KNN], dt.float32, "edrs")
            nc.vector.reciprocal(drs[:], dss[:])
            nc.gpsimd.tensor_tensor(V(dire, ai * 3, [[KNN * 9, 16], [9, KNN], [1, 3]]),
                                    dv[:], V(drs, 0, [[KNN, 16], [1, KNN], [0, 3]]),
                                    ALU.mult)
        nc.sync.dma_start(DV(io['dire'], c * L * KNN * 9,
                             [[KNN * 9, 128], [128 * KNN * 9, 16], [1, KNN * 9]]),
                          V(dire, 0, [[KNN * 9, 16], [1, KNN * 9]]))

        # ---- masks ----
        ctxf = self.t([128, E], dt.float32, "ctxf")
        nc.vector.tensor_tensor(V(ctxf, 0, [[KNN, 16], [1, KNN]]),
                                x0b(A_SEG, 1), rq(12, 1), ALU.is_equal)
        ctx8 = self.t([128, E], dt.uint8, "ctx8")
        nc.vector.tensor_copy(ctx8[:], ctxf[:])
        nc.gpsimd.tensor_scalar(ctxf[:], ctxf[:], -1.0, 1.0, ALU.mult, ALU.add)
        it8 = self.t([128, E], dt.uint8, "it8")
        nc.vector.tensor_copy(it8[:], ctxf[:])
        nc.sync.dma_start(DV(io['ctx'], c * L * KNN,
                             [[KNN, 128], [128 * KNN, 16], [1, KNN]]),
                          V(ctx8, 0, [[KNN, 16], [1, KNN]]))
        nc.sync.dma_start(DV(io['intr'], c * L * KNN,
                             [[KNN, 128], [128 * KNN, 16], [1, KNN]]),
                          V(it8, 0, [[KNN, 16], [1, KNN]]))

        # ---- edge_pos (cos/sin via range-reduced Sin) ----
        dsh = self.t([128, E], dt.uint32, "dsh")
        io9 = self.t([128, 16, KNN], dt.uint32, "io9")
        nc.gpsimd.iota(io9[:], pattern=[[128, 16], [0, KNN]], base=2047,
                       channel_multiplier=1)
        nc.gpsimd.tensor_tensor(V(dsh, 0, [[KNN, 16], [1, KNN]]), io9[:],
                                V(self.col, 0, [[KNN, 16], [1, KNN]]), ALU.subtract)
        epos = self.ht([128, 16, KNN, 16], dt.float32, "epos")
        self.gather_by(
            lambda o, w: V(dsh, o, [[1, w]]),
            lambda o, w: V(epos, o * 16, [[16, w], [1, 16]]),
            DV(io['postable'], 0, [[64, 4095], [1, 16]]),
            E, 16, 64, "wrp")
        nc.sync.dma_start(DV(io['epos'], c * L * KNN * 16,
                             [[KNN * 16, 128], [128 * KNN * 16, 16], [1, KNN * 16]]),
                          V(epos, 0, [[KNN * 16, 16], [1, KNN * 16]]))


_BUILD_CACHE = {}


def _build():
    if 'nc' in _BUILD_CACHE:
        return _BUILD_CACHE['nc'], _BUILD_CACHE['io']
    nc = bacc.Bacc("TRN2", target_bir_lowering=False, debug=True,
                   num_devices=NCORE)
    for val in (EPS, 0.25e-12):
        tns = nc.alloc_sbuf_tensor(f"constap-{val}", [128, 1], dt.float32)
        nc.gpsimd.memset(tns.ap(), val)
        nc.const_aps.aps[(dt.float32, val)] = tns.ap()
    nc.all_engine_barrier()
    io = {}
    io['xplanes'] = nc.dram_tensor("xplanes", [CH, 4, L], dt.float32,
                                   kind="ExternalInput").ap()
    io['xrows'] = nc.dram_tensor("xrows", [CH, L, 16], dt.float32,
                                 kind="ExternalInput").ap()
    io['catab'] = nc.dram_tensor("catab", [CH, L, 64], dt.float32,
                                 kind="ExternalInput").ap()
    io['murow'] = nc.dram_tensor("murow", [128, 16], dt.float32,
                                 kind="ExternalInput").ap()
    io['postable'] = nc.dram_tensor("postable", [4095, 64], dt.float32,
                                    kind="ExternalInput").ap()
    io['angmul'] = nc.dram_tensor("angmul", [CH, 128, 192], dt.float32,
                                  kind="ExternalInput").ap()
    io['angadd'] = nc.dram_tensor("angadd", [CH, 128, 192], dt.float32,
                                  kind="ExternalInput").ap()
    io['framemask'] = nc.dram_tensor("framemask", [128, 16], dt.float32,
                                     kind="ExternalInput").ap()
    io['rectab'] = nc.dram_tensor("rectab", [CH, L, 64], dt.float32).ap()
    for name, shape, d in (
        ('col9', [CH, L, KNN], dt.uint32),
        ('disn', [CH, L, 48], dt.float32),
        ('angn', [CH, L, 12], dt.float32),
        ('dirn', [CH, L, 9], dt.float32),
        ('ctx', [CH, L, KNN], dt.uint8),
        ('intr', [CH, L, KNN], dt.uint8),
        ('epos', [CH, L * KNN, 16], dt.float32),
        ('dise', [CH, 4, L * KNN, 16], dt.float32),
        ('ange', [CH, L * KNN, 4], dt.float32),
        ('dire', [CH, L * KNN, 9], dt.float32),
    ):
        io[name] = nc.dram_tensor(name, shape, d, kind="ExternalOutput").ap()

    with tile.TileContext(nc) as tc:
        with tc.tile_pool(name="sb", bufs=2) as sb, \
             tc.tile_pool(name="big", bufs=2) as big, \
             tc.tile_pool(name="huge", bufs=1) as huge, \
             tc.tile_pool(name="ps", bufs=2, space="PSUM") as ps:
            pools = {'sb': sb, 'big': big, 'huge': huge, 'ps': ps}
            gsem = nc.alloc_semaphore()
            gcnt = [0]
            import os as _os
            nph = int(_os.environ.get("NPHASE", "5"))
            for c in range(CH):
                ch = _Chain(nc, tc, pools, io, c, gsem, gcnt)
                ch.setup()
                ch.node_feats()
                if nph >= 2:
                    ch.topk()
                if nph >= 3:
                    ch.refine()
                if nph >= 4:
                    ch.col_extract()
                if nph >= 5:
                    ch.edge_feats()
    nc.compile()
    _BUILD_CACHE['nc'] = nc
    _BUILD_CACHE['io'] = io
    return nc, io


def _pos_embed_np(pos):
    ang = pos[..., None].astype(np.float32) * POS_FREQ
    return np.concatenate([np.cos(ang), np.sin(ang)], -1).astype(np.float32)


def _make_in_maps(X, seg):
    murow = np.broadcast_to(MUS[None, :], (128, 16)).astype(np.float32).copy()
    framemask = np.ones((128, 16), np.float32)
    for n in (0, 1, 2046, 2047):
        framemask[n % 128, n // 128] = 0.0
    postable = np.zeros((4095, 64), np.float32)
    postable[:, 0:16] = _pos_embed_np(np.arange(-2047, 2048, dtype=np.float32))
    B = 16
    in_maps = []
    for g in range(NCORE):
        xplanes = np.zeros((CH, 4, L), np.float32)
        xrows = np.zeros((CH, L, 16), np.float32)
        catab = np.zeros((CH, L, 64), np.float32)
        angmul = np.ones((CH, 128, 16, 12), np.float32)
        angadd = np.zeros((CH, 128, 16, 12), np.float32)
        for c in range(CH):
            chain = 2 * g + c
            Xc = X[chain * L:(chain + 1) * L]
            sc = seg[chain * L:(chain + 1) * L]
            xplanes[c, 0:3] = Xc[:, 1, :].T
            xplanes[c, 3] = 1.0
            xrows[c, :, 0:12] = Xc.reshape(L, 12)
            xrows[c, :, 12] = sc.astype(np.float32)
            catab[c, :, 0:3] = Xc[:, 1, :]
            for n in (0, 1):
                angmul[c, n % 128, n // 128, :] = 0.0
            n = L - 1
            if chain != B - 1:
                angmul[c, n % 128, n // 128, :] = 0.0
            else:
                angmul[c, n % 128, n // 128, :] = np.array(
                    [1, 0, 0, 1, 0, 0, 1, 0, 0, 1, 0, 0], np.float32)
                angadd[c, n % 128, n // 128, :] = np.array(
                    [0, 1, 1, 0, 0, 0, 0, 1, 1, 0, 0, 0], np.float32)
        in_maps.append({
            "xplanes": xplanes, "xrows": xrows, "catab": catab,
            "murow": murow, "framemask": framemask, "postable": postable,
            "angmul": angmul.reshape(CH, 128, 192),
            "angadd": angadd.reshape(CH, 128, 192),
        })
    return in_maps


def _assemble(r, N):
    col = np.concatenate([r[g]['col9'].reshape(CH * L, KNN) for g in range(NCORE)], 0)
    chain_of = np.arange(N) // L
    col_g = (col.astype(np.int64) + (chain_of * L)[:, None]).astype(np.int32)
    row = np.broadcast_to(np.arange(N, dtype=np.int32)[:, None], (N, KNN))
    edge_index = np.stack([row.reshape(-1).copy(), col_g.reshape(-1)]).astype(np.int32)
    node_pos = np.tile(_pos_embed_np(np.arange(L, dtype=np.float32)), (16, 1))
    cat = lambda k, w: np.concatenate(
        [r[g][k].reshape(-1, w) if w else r[g][k].reshape(-1)
         for g in range(NCORE)], 0)
    dise = np.concatenate(
        [r[g]['dise'].reshape(CH, 4, L * KNN, 16).transpose(0, 2, 1, 3)
         .reshape(CH * L * KNN, 64) for g in range(NCORE)], 0)
    return (node_pos, cat('disn', 48), cat('angn', 12), cat('dirn', 9), edge_index,
            cat('ctx', 0) != 0, cat('intr', 0) != 0, cat('epos', 16),
            dise, cat('ange', 4), cat('dire', 9))


def kernel(X, segment_ids):
    X = np.asarray(X, dtype=np.float32)
    seg = np.asarray(segment_ids, dtype=np.int32)
    N = X.shape[0]
    assert N == 16 * L
    nc, _ = _build()
    in_maps = _make_in_maps(X, seg)
    res = run_bass_kernel_spmd(nc, in_maps, core_ids=list(range(NCORE)))
    return _assemble(res.results, N)
